# revision 15
# baseline (speedup 1.0000x reference)
"""Trainium2 Bass kernel for nn_EstraNet_1443109012284.

Mathematical reduction: the reference's FAVOR+/trig branch (phi_q, aux_q/k,
fr_q/k, aux_A, A) does not feed the output.  The output is exactly

    out[b,n,d] = sum_{h,c} W_o[h,c,d] * norma[h] * sum_{d'} W_v[d',h,c] * x[b,n,d']
               = (x @ M)[b,n,d],   M[d',d] = sum_{h,c} W_v[d',h,c] norma[h] W_o[h,c,d]

with norma[h] = || sum_d s_p[h] W_p[d,h,:] beta_p[d] ||_2.

M is a tiny [512,512] matrix folded on the host; the device does the single
big GEMM  y[32768,512] = x[32768,512] @ M[512,512]  data-parallel over rows:
each of the 8 cores handles 4096 rows.

Device design (per core): compute yT[d, n] = sum_k M[k, d] * xT[k, n]
- lhsT (stationary) = M chunk [128k x 128d]; rhs (moving) = xT quarter
  [128k x 512n], fed pre-transposed from the host (no on-device transpose).
- Same/alternating-weight MMs pipeline at 512/2.4GHz = 216 ns.
- PSUM->SBUF copies all on ONE engine (ACT): PE drain + a single reader
  share PSUM fine; two concurrent readers throttle the PE ~2.3x.
- PE warmed up with dummy matmuls (dep: a memset tile only) during the
  input-DMA window so the HAM clock ramp doesn't tax real work.
- fp16 path (default): x, M, y all fp16, M pre-scaled by an exact power of
  two so M / y avoid the fp16 subnormal range; host multiplies the scale
  back out.  fp16 keeps 10 mantissa bits (vs bf16's 7) and halves output
  DMA vs fp32 -> kernel is PE-bound at ~216ns per [128x128]x[128x512] MM.
"""

import os as _os
import sys

sys.path.insert(0, "/opt/trn_rl_repo")

import numpy as np

import concourse.bass as bass
import concourse.tile as tile
from concourse import bacc, mybir
from concourse.bass_utils import run_bass_kernel_spmd

N_CORES = 8
ROWS = 32768           # B*N = 8*4096
RPC = ROWS // N_CORES  # rows per core = 4096
D = 512
KC = 4                 # contraction chunks of 128
DT = D // 128          # output row-blocks = 4
HB = 4                 # n-quarters per stripe
HW = RPC // HB         # 1024 columns per quarter
JH = HW // 512         # moving chunks of 512 per phase = 2

COMPUTE_DTYPE = _os.environ.get("KERNEL_DTYPE", "fp16")
N_WARM = int(_os.environ.get("KERNEL_NWARM", "8"))

_DT = {
    "fp32": mybir.dt.float32,
    "f32r": mybir.dt.float32r,
    "bf16": mybir.dt.bfloat16,
    "fp16": mybir.dt.float16,
}


def _np_dtype(token):
    if token == "bf16":
        import ml_dtypes

        return ml_dtypes.bfloat16
    if token == "fp16":
        return np.float16
    return np.float32


def _build(token):
    dt_in = _DT[token]
    dt_out = mybir.dt.float16 if token == "fp16" else mybir.dt.float32
    nc = bacc.Bacc("TRN2", target_bir_lowering=False)
    # x pre-transposed, [k-chunk, quarter, 128, 1024]: each quarter-stripe is
    # one contiguous DMA
    xt = nc.dram_tensor("xt", [KC, HB, 128, HW], dt_in, kind="ExternalInput")
    mm = nc.dram_tensor("mm", [128, KC, D], dt_in, kind="ExternalInput")
    yt = nc.dram_tensor("yt", [D, RPC], dt_out, kind="ExternalOutput")

    with tile.TileContext(nc) as tc:
        with (
            tc.tile_pool(name="xp", bufs=1) as xp,
            tc.tile_pool(name="mp", bufs=1) as mp,
            tc.tile_pool(name="op", bufs=4) as op,
            tc.tile_pool(name="pp", bufs=8, space="PSUM") as pp,
        ):
            # PE warmup: matmuls that depend only on a memset tile start at
            # ~6us (right after engine code load) and burn the HAM
            # cold-clock ramp while the x DMAs are still in flight.
            # Always bf16: warmup dtype is independent of the compute dtype
            # (and memset doesn't support float32r).
            wz = mp.tile([128, 512], mybir.dt.bfloat16, name="wz")
            nc.gpsimd.memset(wz[:], 1.0)
            warm = pp.tile([128, 512], mybir.dt.float32, tag="ps", name="warm")
            for w in range(N_WARM):
                nc.tensor.matmul(
                    warm[:], wz[:, 0:128], wz[:], start=True, stop=True
                )

            # Input schedule: the first real matmul needs m(k0) AND x00, so
            # they ride DIFFERENT queues as each queue's FIRST chunk (x00 on
            # sync, m01 on scalar).  The gpsimd SW ring is idle until the
            # first output (~+10us), so it delivers x20/x30 for the stripe-0
            # cadence.  Remaining chunks follow in consumption order.
            m_sb = mp.tile([128, KC, D], dt_in, name="m_sb")
            x_sb = {}
            for h in range(HB):
                for k in range(KC):
                    x_sb[(k, h)] = xp.tile([128, HW], dt_in, tag=f"x{k}{h}", name=f"x{k}{h}")

            def ld(eng, k, h):
                eng.dma_start(out=x_sb[(k, h)][:], in_=xt[k, h])

            # sync queue: x00 first, then k0/k1 of every stripe
            ld(nc.sync, 0, 0)
            ld(nc.sync, 1, 0)
            ld(nc.sync, 0, 1)
            ld(nc.sync, 1, 1)
            ld(nc.sync, 0, 2)
            ld(nc.sync, 1, 2)
            ld(nc.sync, 0, 3)
            ld(nc.sync, 1, 3)
            # scalar queue: m halves first, then k2/k3 of stripes 1-3
            nc.scalar.dma_start(out=m_sb[:, 0:2], in_=mm[:, 0:2])
            nc.scalar.dma_start(out=m_sb[:, 2:4], in_=mm[:, 2:4])
            ld(nc.scalar, 2, 1)
            ld(nc.scalar, 3, 1)
            ld(nc.scalar, 2, 2)
            ld(nc.scalar, 3, 2)
            ld(nc.scalar, 2, 3)
            ld(nc.scalar, 3, 3)
            # gpsimd SW ring: stripe-0 k2/k3 (ring is otherwise idle early)
            ld(nc.gpsimd, 2, 0)
            ld(nc.gpsimd, 3, 0)

            # phases: h outer (first phase only needs the first 4 quarter
            # DMAs), d inner.  k-major MM order (4 weight switches per
            # phase, banks finish staggered); last phase j-major with per-
            # bank copy+DMA so the tail is short.
            NPH = HB * DT
            for ph in range(NPH):
                h, d = divmod(ph, DT)
                d0 = d * 128
                last = ph == NPH - 1
                ot = op.tile([128, HW], dt_out, name=f"ot{ph}", tag="ot")
                pss = [
                    pp.tile([128, 512], mybir.dt.float32, tag="ps", name=f"ps_{h}_{d}_{j}")
                    for j in range(JH)
                ]
                # alternate output DMAs between the sync HWDGE queue and the
                # gpsimd SWDGE rings (POOL sequencer is otherwise idle) so
                # input and output streams don't serialize on one ring.
                # Scalar's sequencer is copy-only: a DMA issue between copies
                # delays the PSUM drain and back-pressures the PE.
                oeng = nc.gpsimd if ph % 2 == 0 else nc.sync
                if last:
                    # final phase: quarter-granularity copies, each quarter's
                    # store on a DIFFERENT engine's queue (sync x2, gpsimd,
                    # scalar-as-its-last-instruction) so the four issues
                    # don't serialize on one sequencer after the last MM
                    q_eng = [nc.sync, nc.gpsimd, nc.sync, None]
                    tail_dma = []
                    for j in range(JH):
                        for k in range(KC):
                            nc.tensor.matmul(
                                pss[j][:],
                                m_sb[:, k, d0 : d0 + 128],
                                x_sb[(k, h)][:, j * 512 : (j + 1) * 512],
                                start=(k == 0),
                                stop=(k == KC - 1),
                            )
                        for q in range(2):
                            c0 = j * 512 + q * 256
                            nc.scalar.copy(ot[:, c0 : c0 + 256], pss[j][:, q * 256 : (q + 1) * 256])
                            eng = q_eng[j * 2 + q]
                            args = dict(
                                out=yt[d0 : d0 + 128, h * HW + c0 : h * HW + c0 + 256],
                                in_=ot[:, c0 : c0 + 256],
                            )
                            if eng is None:
                                tail_dma.append(args)  # issue on ACT after all copies
                            else:
                                eng.dma_start(**args)
                    for args in tail_dma:
                        nc.scalar.dma_start(**args)
                else:
                    for k in range(KC):
                        for j in range(JH):
                            nc.tensor.matmul(
                                pss[j][:],
                                m_sb[:, k, d0 : d0 + 128],
                                x_sb[(k, h)][:, j * 512 : (j + 1) * 512],
                                start=(k == 0),
                                stop=(k == KC - 1),
                            )
                    for j in range(JH):
                        nc.scalar.copy(ot[:, j * 512 : (j + 1) * 512], pss[j][:])
                    oeng.dma_start(
                        out=yt[d0 : d0 + 128, h * HW : (h + 1) * HW], in_=ot[:]
                    )
    nc.compile()
    return nc


def _fold_m(W_v, s_p, W_p, beta_p, W_o):
    """Host-side constant folding of the tiny parameter tensors into M."""
    W_v = np.asarray(W_v, dtype=np.float64)
    s_p = np.asarray(s_p, dtype=np.float64)
    W_p = np.asarray(W_p, dtype=np.float64)
    beta_p = np.asarray(beta_p, dtype=np.float64)
    W_o = np.asarray(W_o, dtype=np.float64)
    phi = np.einsum("h,dhc,d->hc", s_p, W_p, beta_p)
    norma = np.linalg.norm(phi, axis=1)  # [h]
    M = np.einsum("dhc,h,hce->de", W_v, norma, W_o)  # [512, 512]
    return M.astype(np.float32)


_prog_cache = {}
_last_in_maps = None  # kept for test.py profiling reuse
_last_result = None


def _run(in_maps, token, **kwargs):
    if token not in _prog_cache:
        _prog_cache[token] = _build(token)
    return run_bass_kernel_spmd(_prog_cache[token], in_maps, list(range(N_CORES)), **kwargs)


def kernel(x, W_v, s_p, c_p, W_p, W_A, W_o, beta_p, beta_i_p, **_unused):
    global _last_in_maps, _last_result
    token = COMPUTE_DTYPE
    np_dt = _np_dtype(token)

    x = np.asarray(x, dtype=np.float32)
    M = _fold_m(W_v, s_p, W_p, beta_p, W_o)

    # fp16 path: scale M by an exact power of two so M entries and y values
    # sit in fp16 normal range; undo on the host after the run
    out_unscale = 1.0
    if token == "fp16":
        amax = float(np.abs(M).max())
        if amax > 0:
            e = int(np.floor(-np.log2(amax)))
            M = M * np.float32(2.0**e)
            out_unscale = 2.0**-e

    B, N, Dd = x.shape
    assert B * N == ROWS and Dd == D, (x.shape,)

    mmc = np.ascontiguousarray(M.reshape(KC, 128, D).transpose(1, 0, 2)).astype(np_dt)
    xf = x.reshape(ROWS, D)

    in_maps = []
    for c in range(N_CORES):
        sh = xf[c * RPC : (c + 1) * RPC]               # [4096, 512]
        xT = sh.T.astype(np_dt)                        # [512, 4096]
        # [KC, 128, HB, HW] -> [KC, HB, 128, HW], each quarter contiguous
        xs = np.ascontiguousarray(
            xT.reshape(KC, 128, HB, HW).transpose(0, 2, 1, 3)
        )
        in_maps.append({"xt": xs, "mm": mmc})

    _last_in_maps = in_maps
    res = _run(in_maps, token)
    _last_result = res
    out = np.empty((ROWS, D), dtype=np.float32)
    for c in range(N_CORES):
        yc = res.results[c]["yt"].astype(np.float32)
        if out_unscale != 1.0:
            yc *= np.float32(out_unscale)
        out[c * RPC : (c + 1) * RPC] = yc.T
    return out.reshape(B, N, D)


if __name__ == "__main__":
    # smoke test with random data
    rng = np.random.default_rng(0)
    x = rng.standard_normal((8, 4096, 512)).astype(np.float32)
    W_v = rng.standard_normal((512, 8, 64)).astype(np.float32) * 0.01
    s_p = np.ones((8,), np.float32)
    c_p = np.ones((8,), np.float32)
    W_p = rng.standard_normal((512, 8, 64)).astype(np.float32) * 0.01
    W_A = rng.standard_normal((256, 64)).astype(np.float32)
    W_o = rng.standard_normal((8, 64, 512)).astype(np.float32) * 0.01
    beta_p = rng.standard_normal((512,)).astype(np.float32) * 1e-5
    beta_i_p = rng.standard_normal((4096, 512)).astype(np.float32) * 1e-5
    out = kernel(x, W_v=W_v, s_p=s_p, c_p=c_p, W_p=W_p, W_A=W_A, W_o=W_o,
                 beta_p=beta_p, beta_i_p=beta_i_p)
    M = _fold_m(W_v, s_p, W_p, beta_p, W_o)
    exp = (x.reshape(-1, 512).astype(np.float64) @ M.astype(np.float64)).reshape(8, 4096, 512)
    err = np.abs(out - exp).max() / (np.abs(exp).max() + 1e-30)
    print("smoke rel err:", err)



# revision 25
# speedup vs baseline: 1.0002x; 1.0002x over previous
"""Trainium2 Bass kernel for nn_EstraNet_1443109012284.

Mathematical reduction: the reference's FAVOR+/trig branch (phi_q, aux_q/k,
fr_q/k, aux_A, A) does not feed the output.  The output is exactly

    out[b,n,d] = sum_{h,c} W_o[h,c,d] * norma[h] * sum_{d'} W_v[d',h,c] * x[b,n,d']
               = (x @ M)[b,n,d],   M[d',d] = sum_{h,c} W_v[d',h,c] norma[h] W_o[h,c,d]

with norma[h] = || sum_d s_p[h] W_p[d,h,:] beta_p[d] ||_2.

M is a tiny [512,512] matrix folded on the host; the device does the single
big GEMM  y[32768,512] = x[32768,512] @ M[512,512]  data-parallel over rows:
each of the 8 cores handles 4096 rows.

Device design (per core): compute yT[d, n] = sum_k M[k, d] * xT[k, n]
- lhsT (stationary) = M chunk [128k x 128d]; rhs (moving) = xT quarter
  [128k x 512n], fed pre-transposed from the host (no on-device transpose).
- Same/alternating-weight MMs pipeline at 512/2.4GHz = 216 ns.
- PSUM->SBUF copies all on ONE engine (ACT): PE drain + a single reader
  share PSUM fine; two concurrent readers throttle the PE ~2.3x.
- fp16 path (default): x, M, y all fp16, M pre-scaled by an exact power of
  two so M / y avoid the fp16 subnormal range; host multiplies the scale
  back out.  fp16 keeps 10 mantissa bits (vs bf16's 7) and halves output
  DMA vs fp32 -> kernel is PE-bound at ~216ns per [128x128]x[128x512] MM.

Measured time structure (NTFF profile, exec window = first const memset to
last epilogue instruction): ~2.4us to the first warmup MM (engine preamble
+ memset dep), ~5.5us of warmup MMs covering the PE half-clock p-state
ramp AND the first input chunks' DMA latency, 128 real MMs at 216ns
(27.7us), ~3.2us output drain after the last MM, and ~9us of fixed
framework epilogue (DMA-sem waits + barriers + a 256-semaphore clear that
the NEFF lowering appends, ~6.5us, not controllable from kernel code).

Tuning knobs (all measured against heavy run-to-run DMA-bandwidth
variance: per-queue early input rate swings 65-133 GB/s with co-tenant
load; the ~360 GB/s DMA-engine fabric is shared round-robin by all
active queues):
- N_WARM=14: warmups end ~+7.9us; the first real MM then never waits on
  the m/x DMAs even in slow-supply runs.  Fewer warmups lower the floor
  by ~0.2us but add a ~40% tail mode of +2..3.5us input stalls.
- m is split in halves, one leading each HWDGE queue, so the first real
  MM waits on a 256KB chunk, not 512KB serialized ahead of the x stream.
- Output DMAs alternate gpsimd-SW/sync-HW per phase; the ACT sequencer
  is copy-only (a DMA issue between copies delays the PSUM drain and
  back-pressures the PE through bank reuse).
- Last phase (TAIL=S) runs j-major: the first 512-col half drains on
  gpsimd while the second computes; the final half drains as two 256-col
  chunks on sync + scalar in parallel (drain 3.2us vs 3.5us for the
  single-queue quarter scheme).
"""

import os as _os
import sys

sys.path.insert(0, "/opt/trn_rl_repo")

import numpy as np

import concourse.bass as bass
import concourse.tile as tile
from concourse import bacc, mybir
from concourse.bass_utils import run_bass_kernel_spmd

N_CORES = 8
ROWS = 32768           # B*N = 8*4096
RPC = ROWS // N_CORES  # rows per core = 4096
D = 512
KC = 4                 # contraction chunks of 128
DT = D // 128          # output row-blocks = 4
HB = 4                 # n-quarters per stripe
HW = RPC // HB         # 1024 columns per quarter
JH = HW // 512         # moving chunks of 512 per phase = 2

COMPUTE_DTYPE = _os.environ.get("KERNEL_DTYPE", "fp16")
N_WARM = int(_os.environ.get("KERNEL_NWARM", "14"))
SCHED = _os.environ.get("KERNEL_SCHED", "A")  # input queue schedule variant
TAIL = _os.environ.get("KERNEL_TAIL", "S")   # S=split last phase, Q=quarter DMAs
FLAGS = _os.environ.get("KERNEL_FLAGS", "")  # "na" = no asserts/race detect

_DT = {
    "fp32": mybir.dt.float32,
    "f32r": mybir.dt.float32r,
    "bf16": mybir.dt.bfloat16,
    "fp16": mybir.dt.float16,
}


def _np_dtype(token):
    if token == "bf16":
        import ml_dtypes

        return ml_dtypes.bfloat16
    if token == "fp16":
        return np.float16
    return np.float32


def _build(token):
    dt_in = _DT[token]
    dt_out = mybir.dt.float16 if token == "fp16" else mybir.dt.float32
    kw = {}
    if "na" in FLAGS:
        kw = dict(enable_asserts=False, detect_race_conditions=False)
    nc = bacc.Bacc("TRN2", target_bir_lowering=False, **kw)
    # x pre-transposed, [k-chunk, quarter, 128, 1024]: each quarter-stripe is
    # one contiguous DMA
    xt = nc.dram_tensor("xt", [KC, HB, 128, HW], dt_in, kind="ExternalInput")
    mm = nc.dram_tensor("mm", [128, KC, D], dt_in, kind="ExternalInput")
    yt = nc.dram_tensor("yt", [D, RPC], dt_out, kind="ExternalOutput")

    with tile.TileContext(nc) as tc:
        with (
            tc.tile_pool(name="xp", bufs=1) as xp,
            tc.tile_pool(name="mp", bufs=1) as mp,
            tc.tile_pool(name="op", bufs=4) as op,
            tc.tile_pool(name="pp", bufs=8, space="PSUM") as pp,
        ):
            # PE warmup: matmuls that depend only on a memset tile start at
            # ~6us (right after engine code load) and burn the HAM
            # cold-clock ramp while the x DMAs are still in flight.
            # Always bf16: warmup dtype is independent of the compute dtype
            # (and memset doesn't support float32r).
            wz = mp.tile([128, 512], mybir.dt.bfloat16, name="wz")
            nc.gpsimd.memset(wz[:], 1.0)
            warm = pp.tile([128, 512], mybir.dt.float32, tag="ps", name="warm")
            for w in range(N_WARM):
                nc.tensor.matmul(
                    warm[:], wz[:, 0:128], wz[:], start=True, stop=True
                )

            # Input schedule: the first real matmul needs m(k0) AND x00, so
            # they ride DIFFERENT queues as each queue's FIRST chunk (x00 on
            # sync, m01 on scalar).  The gpsimd SW ring is idle until the
            # first output (~+10us), so it delivers x20/x30 for the stripe-0
            # cadence.  Remaining chunks follow in consumption order.
            m_sb = mp.tile([128, KC, D], dt_in, name="m_sb")
            x_sb = {}
            for h in range(HB):
                for k in range(KC):
                    x_sb[(k, h)] = xp.tile([128, HW], dt_in, tag=f"x{k}{h}", name=f"x{k}{h}")

            def ld(eng, k, h):
                eng.dma_start(out=x_sb[(k, h)][:], in_=xt[k, h])

            if SCHED == "C":
                # deadline-robust: x00/m01 lead different queues; gpsimd's
                # idle early window carries x10/x30/x31; every chunk lands
                # with >=1us of margin at measured ~110GB/s per queue
                for k, h in ((0, 0), (2, 0), (0, 1), (2, 1), (0, 2), (2, 2), (0, 3), (2, 3)):
                    ld(nc.sync, k, h)
                nc.scalar.dma_start(out=m_sb[:, 0:2], in_=mm[:, 0:2])
                nc.scalar.dma_start(out=m_sb[:, 2:4], in_=mm[:, 2:4])
                for k, h in ((1, 1), (1, 2), (3, 2), (1, 3), (3, 3)):
                    ld(nc.scalar, k, h)
                for k, h in ((1, 0), (3, 0), (3, 1)):
                    ld(nc.gpsimd, k, h)
            elif SCHED == "B":
                # sync queue: x00 first, then k0/k1 of every stripe
                for h in range(HB):
                    ld(nc.sync, 0, h)
                    ld(nc.sync, 1, h)
                # scalar queue: m halves first, then k2/k3 of stripes 1-3
                nc.scalar.dma_start(out=m_sb[:, 0:2], in_=mm[:, 0:2])
                nc.scalar.dma_start(out=m_sb[:, 2:4], in_=mm[:, 2:4])
                for h in range(1, HB):
                    ld(nc.scalar, 2, h)
                    ld(nc.scalar, 3, h)
                # gpsimd SW ring: stripe-0 k2/k3 (ring is otherwise idle early)
                ld(nc.gpsimd, 2, 0)
                ld(nc.gpsimd, 3, 0)
            else:
                # "A" (v6): m halves lead each HW queue, x alternates
                nc.sync.dma_start(out=m_sb[:, 0:2], in_=mm[:, 0:2])
                nc.scalar.dma_start(out=m_sb[:, 2:4], in_=mm[:, 2:4])
                for h in range(HB):
                    for k in range(KC):
                        eng = nc.sync if (h * KC + k) % 2 == 0 else nc.scalar
                        ld(eng, k, h)

            # phases: h outer (first phase only needs the first 4 quarter
            # DMAs), d inner.  k-major MM order (4 weight switches per
            # phase, banks finish staggered); last phase j-major with per-
            # bank copy+DMA so the tail is short.
            NPH = HB * DT
            for ph in range(NPH):
                h, d = divmod(ph, DT)
                d0 = d * 128
                last = ph == NPH - 1
                ot = op.tile([128, HW], dt_out, name=f"ot{ph}", tag="ot")
                pss = [
                    pp.tile([128, 512], mybir.dt.float32, tag="ps", name=f"ps_{h}_{d}_{j}")
                    for j in range(JH)
                ]
                # alternate output DMAs between the sync HWDGE queue and the
                # gpsimd SWDGE rings (POOL sequencer is otherwise idle) so
                # input and output streams don't serialize on one ring.
                # Scalar's sequencer is copy-only: a DMA issue between copies
                # delays the PSUM drain and back-pressures the PE.
                oeng = nc.gpsimd if ph % 2 == 0 else nc.sync
                if last and TAIL == "V":
                    # j0 drains as one 512 chunk on gpsimd; j1's two 256-col
                    # copies run CONCURRENTLY on ACT and DVE (PE is done, so
                    # a second PSUM reader can't throttle it), stores on
                    # sync + scalar in parallel
                    for k in range(KC):
                        nc.tensor.matmul(
                            pss[0][:],
                            m_sb[:, k, d0 : d0 + 128],
                            x_sb[(k, h)][:, 0:512],
                            start=(k == 0),
                            stop=(k == KC - 1),
                        )
                    nc.scalar.copy(ot[:, 0:512], pss[0][:])
                    nc.gpsimd.dma_start(
                        out=yt[d0 : d0 + 128, h * HW : h * HW + 512],
                        in_=ot[:, 0:512],
                    )
                    for k in range(KC):
                        nc.tensor.matmul(
                            pss[1][:],
                            m_sb[:, k, d0 : d0 + 128],
                            x_sb[(k, h)][:, 512:1024],
                            start=(k == 0),
                            stop=(k == KC - 1),
                        )
                    nc.vector.tensor_copy(out=ot[:, 768:1024], in_=pss[1][:, 256:512])
                    nc.scalar.copy(ot[:, 512:768], pss[1][:, 0:256])
                    nc.sync.dma_start(
                        out=yt[d0 : d0 + 128, h * HW + 512 : h * HW + 768],
                        in_=ot[:, 512:768],
                    )
                    nc.scalar.dma_start(
                        out=yt[d0 : d0 + 128, h * HW + 768 : h * HW + 1024],
                        in_=ot[:, 768:1024],
                    )
                elif last and TAIL == "S":
                    # split final phase: j0 drains as one 512-col chunk on
                    # gpsimd while j1 computes; j1 ends with two 256-col
                    # copies whose stores ride sync + scalar in parallel
                    for k in range(KC):
                        nc.tensor.matmul(
                            pss[0][:],
                            m_sb[:, k, d0 : d0 + 128],
                            x_sb[(k, h)][:, 0:512],
                            start=(k == 0),
                            stop=(k == KC - 1),
                        )
                    nc.scalar.copy(ot[:, 0:512], pss[0][:])
                    nc.gpsimd.dma_start(
                        out=yt[d0 : d0 + 128, h * HW : h * HW + 512],
                        in_=ot[:, 0:512],
                    )
                    for k in range(KC):
                        nc.tensor.matmul(
                            pss[1][:],
                            m_sb[:, k, d0 : d0 + 128],
                            x_sb[(k, h)][:, 512:1024],
                            start=(k == 0),
                            stop=(k == KC - 1),
                        )
                    nc.scalar.copy(ot[:, 512:768], pss[1][:, 0:256])
                    nc.sync.dma_start(
                        out=yt[d0 : d0 + 128, h * HW + 512 : h * HW + 768],
                        in_=ot[:, 512:768],
                    )
                    nc.scalar.copy(ot[:, 768:1024], pss[1][:, 256:512])
                    nc.scalar.dma_start(
                        out=yt[d0 : d0 + 128, h * HW + 768 : h * HW + 1024],
                        in_=ot[:, 768:1024],
                    )
                elif last:
                    # final phase: quarter-granularity copies, each quarter's
                    # store on a DIFFERENT engine's queue (sync x2, gpsimd,
                    # scalar-as-its-last-instruction) so the four issues
                    # don't serialize on one sequencer after the last MM
                    q_eng = [nc.sync, nc.gpsimd, nc.sync, None]
                    tail_dma = []
                    for j in range(JH):
                        for k in range(KC):
                            nc.tensor.matmul(
                                pss[j][:],
                                m_sb[:, k, d0 : d0 + 128],
                                x_sb[(k, h)][:, j * 512 : (j + 1) * 512],
                                start=(k == 0),
                                stop=(k == KC - 1),
                            )
                        for q in range(2):
                            c0 = j * 512 + q * 256
                            nc.scalar.copy(ot[:, c0 : c0 + 256], pss[j][:, q * 256 : (q + 1) * 256])
                            eng = q_eng[j * 2 + q]
                            args = dict(
                                out=yt[d0 : d0 + 128, h * HW + c0 : h * HW + c0 + 256],
                                in_=ot[:, c0 : c0 + 256],
                            )
                            if eng is None:
                                tail_dma.append(args)  # issue on ACT after all copies
                            else:
                                eng.dma_start(**args)
                    for args in tail_dma:
                        nc.scalar.dma_start(**args)
                else:
                    for k in range(KC):
                        for j in range(JH):
                            nc.tensor.matmul(
                                pss[j][:],
                                m_sb[:, k, d0 : d0 + 128],
                                x_sb[(k, h)][:, j * 512 : (j + 1) * 512],
                                start=(k == 0),
                                stop=(k == KC - 1),
                            )
                    for j in range(JH):
                        nc.scalar.copy(ot[:, j * 512 : (j + 1) * 512], pss[j][:])
                    oeng.dma_start(
                        out=yt[d0 : d0 + 128, h * HW : (h + 1) * HW], in_=ot[:]
                    )
    nc.compile()
    return nc


def _fold_m(W_v, s_p, W_p, beta_p, W_o):
    """Host-side constant folding of the tiny parameter tensors into M."""
    W_v = np.asarray(W_v, dtype=np.float64)
    s_p = np.asarray(s_p, dtype=np.float64)
    W_p = np.asarray(W_p, dtype=np.float64)
    beta_p = np.asarray(beta_p, dtype=np.float64)
    W_o = np.asarray(W_o, dtype=np.float64)
    phi = np.einsum("h,dhc,d->hc", s_p, W_p, beta_p)
    norma = np.linalg.norm(phi, axis=1)  # [h]
    M = np.einsum("dhc,h,hce->de", W_v, norma, W_o)  # [512, 512]
    return M.astype(np.float32)


_prog_cache = {}
_last_in_maps = None  # kept for test.py profiling reuse
_last_result = None


def _run(in_maps, token, **kwargs):
    if token not in _prog_cache:
        _prog_cache[token] = _build(token)
    return run_bass_kernel_spmd(_prog_cache[token], in_maps, list(range(N_CORES)), **kwargs)


def kernel(x, W_v, s_p, c_p, W_p, W_A, W_o, beta_p, beta_i_p, **_unused):
    global _last_in_maps, _last_result
    token = COMPUTE_DTYPE
    np_dt = _np_dtype(token)

    x = np.asarray(x, dtype=np.float32)
    M = _fold_m(W_v, s_p, W_p, beta_p, W_o)

    # fp16 path: scale M by an exact power of two so M entries and y values
    # sit in fp16 normal range; undo on the host after the run
    out_unscale = 1.0
    if token == "fp16":
        amax = float(np.abs(M).max())
        if amax > 0:
            e = int(np.floor(-np.log2(amax)))
            M = M * np.float32(2.0**e)
            out_unscale = 2.0**-e

    B, N, Dd = x.shape
    assert B * N == ROWS and Dd == D, (x.shape,)

    mmc = np.ascontiguousarray(M.reshape(KC, 128, D).transpose(1, 0, 2)).astype(np_dt)
    xf = x.reshape(ROWS, D)

    in_maps = []
    for c in range(N_CORES):
        sh = xf[c * RPC : (c + 1) * RPC]               # [4096, 512]
        xT = sh.T.astype(np_dt)                        # [512, 4096]
        # [KC, 128, HB, HW] -> [KC, HB, 128, HW], each quarter contiguous
        xs = np.ascontiguousarray(
            xT.reshape(KC, 128, HB, HW).transpose(0, 2, 1, 3)
        )
        in_maps.append({"xt": xs, "mm": mmc})

    _last_in_maps = in_maps
    res = _run(in_maps, token)
    _last_result = res
    out = np.empty((ROWS, D), dtype=np.float32)
    for c in range(N_CORES):
        yc = res.results[c]["yt"].astype(np.float32)
        if out_unscale != 1.0:
            yc *= np.float32(out_unscale)
        out[c * RPC : (c + 1) * RPC] = yc.T
    return out.reshape(B, N, D)


if __name__ == "__main__":
    # smoke test with random data
    rng = np.random.default_rng(0)
    x = rng.standard_normal((8, 4096, 512)).astype(np.float32)
    W_v = rng.standard_normal((512, 8, 64)).astype(np.float32) * 0.01
    s_p = np.ones((8,), np.float32)
    c_p = np.ones((8,), np.float32)
    W_p = rng.standard_normal((512, 8, 64)).astype(np.float32) * 0.01
    W_A = rng.standard_normal((256, 64)).astype(np.float32)
    W_o = rng.standard_normal((8, 64, 512)).astype(np.float32) * 0.01
    beta_p = rng.standard_normal((512,)).astype(np.float32) * 1e-5
    beta_i_p = rng.standard_normal((4096, 512)).astype(np.float32) * 1e-5
    out = kernel(x, W_v=W_v, s_p=s_p, c_p=c_p, W_p=W_p, W_A=W_A, W_o=W_o,
                 beta_p=beta_p, beta_i_p=beta_i_p)
    M = _fold_m(W_v, s_p, W_p, beta_p, W_o)
    exp = (x.reshape(-1, 512).astype(np.float64) @ M.astype(np.float64)).reshape(8, 4096, 512)
    err = np.abs(out - exp).max() / (np.abs(exp).max() + 1e-30)
    print("smoke rel err:", err)



# revision 26
# speedup vs baseline: 1.0067x; 1.0065x over previous
"""Trainium2 Bass kernel for nn_EstraNet_1443109012284.

Mathematical reduction: the reference's FAVOR+/trig branch (phi_q, aux_q/k,
fr_q/k, aux_A, A) does not feed the output.  The output is exactly

    out[b,n,d] = sum_{h,c} W_o[h,c,d] * norma[h] * sum_{d'} W_v[d',h,c] * x[b,n,d']
               = (x @ M)[b,n,d],   M[d',d] = sum_{h,c} W_v[d',h,c] norma[h] W_o[h,c,d]

with norma[h] = || sum_d s_p[h] W_p[d,h,:] beta_p[d] ||_2.

M is a tiny [512,512] matrix folded on the host; the device does the single
big GEMM  y[32768,512] = x[32768,512] @ M[512,512]  data-parallel over rows:
each of the 8 cores handles 4096 rows.

Device design (per core): compute yT[d, n] = sum_k M[k, d] * xT[k, n]
- lhsT (stationary) = M chunk [128k x 128d]; rhs (moving) = xT quarter
  [128k x 512n], fed pre-transposed from the host (no on-device transpose).
- Same/alternating-weight MMs pipeline at 512/2.4GHz = 216 ns.
- PSUM->SBUF copies all on ONE engine (ACT): PE drain + a single reader
  share PSUM fine; two concurrent readers throttle the PE ~2.3x.
- fp16 path (default): x, M, y all fp16, M pre-scaled by an exact power of
  two so M / y avoid the fp16 subnormal range; host multiplies the scale
  back out.  fp16 keeps 10 mantissa bits (vs bf16's 7) and halves output
  DMA vs fp32 -> kernel is PE-bound at ~216ns per [128x128]x[128x512] MM.

Measured time structure (NTFF profile, exec window = first const memset to
last epilogue instruction): ~2.4us to the first warmup MM (engine preamble
+ memset dep), ~5.5us of warmup MMs covering the PE half-clock p-state
ramp AND the first input chunks' DMA latency, 128 real MMs at 216ns
(27.7us), ~3.2us output drain after the last MM, and ~9us of fixed
framework epilogue (DMA-sem waits + barriers + a 256-semaphore clear that
the NEFF lowering appends, ~6.5us, not controllable from kernel code).

Tuning knobs (all measured against heavy run-to-run DMA-bandwidth
variance: per-queue early input rate swings 65-133 GB/s with co-tenant
load; the ~360 GB/s DMA-engine fabric is shared round-robin by all
active queues):
- N_WARM=14: warmups end ~+7.9us; the first real MM then never waits on
  the m/x DMAs even in slow-supply runs.  Fewer warmups lower the floor
  by ~0.2us but add a ~40% tail mode of +2..3.5us input stalls.
- m is split in halves, one leading each HWDGE queue, so the first real
  MM waits on a 256KB chunk, not 512KB serialized ahead of the x stream.
- Output DMAs alternate gpsimd-SW/sync-HW per phase; the ACT sequencer
  is copy-only (a DMA issue between copies delays the PSUM drain and
  back-pressures the PE through bank reuse).
- Last phase (TAIL=S) runs j-major: the first 512-col half drains on
  gpsimd while the second computes; the final half drains as two 256-col
  chunks on sync + scalar in parallel (drain 3.2us vs 3.5us for the
  single-queue quarter scheme).
"""

import os as _os
import sys

sys.path.insert(0, "/opt/trn_rl_repo")

import numpy as np

import concourse.bass as bass
import concourse.tile as tile
from concourse import bacc, mybir
from concourse.bass_utils import run_bass_kernel_spmd

N_CORES = 8
ROWS = 32768           # B*N = 8*4096
RPC = ROWS // N_CORES  # rows per core = 4096
D = 512
KC = 4                 # contraction chunks of 128
DT = D // 128          # output row-blocks = 4
HB = 4                 # n-quarters per stripe
HW = RPC // HB         # 1024 columns per quarter
JH = HW // 512         # moving chunks of 512 per phase = 2

COMPUTE_DTYPE = _os.environ.get("KERNEL_DTYPE", "fp16")
N_WARM = int(_os.environ.get("KERNEL_NWARM", "14"))
SCHED = _os.environ.get("KERNEL_SCHED", "A")  # input queue schedule variant
TAIL = _os.environ.get("KERNEL_TAIL", "S")   # S=split last phase, Q=quarter DMAs
FLAGS = _os.environ.get("KERNEL_FLAGS", "")  # "na" = no asserts/race detect

_DT = {
    "fp32": mybir.dt.float32,
    "f32r": mybir.dt.float32r,
    "bf16": mybir.dt.bfloat16,
    "fp16": mybir.dt.float16,
}


def _np_dtype(token):
    if token == "bf16":
        import ml_dtypes

        return ml_dtypes.bfloat16
    if token == "fp16":
        return np.float16
    return np.float32


def _build(token):
    dt_in = _DT[token]
    dt_out = mybir.dt.float16 if token == "fp16" else mybir.dt.float32
    kw = {}
    if "na" in FLAGS:
        kw = dict(enable_asserts=False, detect_race_conditions=False)
    nc = bacc.Bacc("TRN2", target_bir_lowering=False, **kw)
    # x pre-transposed, [k-chunk, quarter, 128, 1024]: each quarter-stripe is
    # one contiguous DMA
    xt = nc.dram_tensor("xt", [KC, HB, 128, HW], dt_in, kind="ExternalInput")
    mm = nc.dram_tensor("mm", [128, KC, D], dt_in, kind="ExternalInput")
    yt = nc.dram_tensor("yt", [D, RPC], dt_out, kind="ExternalOutput")

    with tile.TileContext(nc) as tc:
        with (
            tc.tile_pool(name="xp", bufs=1) as xp,
            tc.tile_pool(name="mp", bufs=1) as mp,
            tc.tile_pool(name="op", bufs=4) as op,
            tc.tile_pool(name="pp", bufs=8, space="PSUM") as pp,
        ):
            # PE warmup: matmuls that depend only on a memset tile start at
            # ~6us (right after engine code load) and burn the HAM
            # cold-clock ramp while the x DMAs are still in flight.
            # Always bf16: warmup dtype is independent of the compute dtype
            # (and memset doesn't support float32r).
            wz = mp.tile([128, 512], mybir.dt.bfloat16, name="wz")
            nc.gpsimd.memset(wz[:], 1.0)
            warm = pp.tile([128, 512], mybir.dt.float32, tag="ps", name="warm")
            for w in range(N_WARM):
                nc.tensor.matmul(
                    warm[:], wz[:, 0:128], wz[:], start=True, stop=True
                )

            # Input schedule: the first real matmul needs m(k0) AND x00, so
            # they ride DIFFERENT queues as each queue's FIRST chunk (x00 on
            # sync, m01 on scalar).  The gpsimd SW ring is idle until the
            # first output (~+10us), so it delivers x20/x30 for the stripe-0
            # cadence.  Remaining chunks follow in consumption order.
            m_sb = mp.tile([128, KC, D], dt_in, name="m_sb")
            x_sb = {}
            for h in range(HB):
                for k in range(KC):
                    x_sb[(k, h)] = xp.tile([128, HW], dt_in, tag=f"x{k}{h}", name=f"x{k}{h}")

            def ld(eng, k, h):
                eng.dma_start(out=x_sb[(k, h)][:], in_=xt[k, h])

            if SCHED == "D":
                # x00 leads sync (no m ahead of it); m01 rides the idle SW
                # ring; m23 leads scalar.  Good-window data-ready drops from
                # ~+6.1us to ~+4.7us, so fewer warmups are needed.
                for k, h in ((0, 0), (0, 1), (1, 1), (0, 2), (1, 2), (0, 3), (1, 3)):
                    ld(nc.sync, k, h)
                nc.scalar.dma_start(out=m_sb[:, 2:4], in_=mm[:, 2:4])
                for k, h in ((1, 0), (2, 1), (3, 1), (2, 2), (3, 2), (2, 3), (3, 3)):
                    ld(nc.scalar, k, h)
                nc.gpsimd.dma_start(out=m_sb[:, 0:2], in_=mm[:, 0:2])
                ld(nc.gpsimd, 2, 0)
                ld(nc.gpsimd, 3, 0)
            elif SCHED == "C":
                # deadline-robust: x00/m01 lead different queues; gpsimd's
                # idle early window carries x10/x30/x31; every chunk lands
                # with >=1us of margin at measured ~110GB/s per queue
                for k, h in ((0, 0), (2, 0), (0, 1), (2, 1), (0, 2), (2, 2), (0, 3), (2, 3)):
                    ld(nc.sync, k, h)
                nc.scalar.dma_start(out=m_sb[:, 0:2], in_=mm[:, 0:2])
                nc.scalar.dma_start(out=m_sb[:, 2:4], in_=mm[:, 2:4])
                for k, h in ((1, 1), (1, 2), (3, 2), (1, 3), (3, 3)):
                    ld(nc.scalar, k, h)
                for k, h in ((1, 0), (3, 0), (3, 1)):
                    ld(nc.gpsimd, k, h)
            elif SCHED == "B":
                # sync queue: x00 first, then k0/k1 of every stripe
                for h in range(HB):
                    ld(nc.sync, 0, h)
                    ld(nc.sync, 1, h)
                # scalar queue: m halves first, then k2/k3 of stripes 1-3
                nc.scalar.dma_start(out=m_sb[:, 0:2], in_=mm[:, 0:2])
                nc.scalar.dma_start(out=m_sb[:, 2:4], in_=mm[:, 2:4])
                for h in range(1, HB):
                    ld(nc.scalar, 2, h)
                    ld(nc.scalar, 3, h)
                # gpsimd SW ring: stripe-0 k2/k3 (ring is otherwise idle early)
                ld(nc.gpsimd, 2, 0)
                ld(nc.gpsimd, 3, 0)
            else:
                # "A" (v6): m halves lead each HW queue, x alternates
                nc.sync.dma_start(out=m_sb[:, 0:2], in_=mm[:, 0:2])
                nc.scalar.dma_start(out=m_sb[:, 2:4], in_=mm[:, 2:4])
                for h in range(HB):
                    for k in range(KC):
                        eng = nc.sync if (h * KC + k) % 2 == 0 else nc.scalar
                        ld(eng, k, h)

            # phases: h outer (first phase only needs the first 4 quarter
            # DMAs), d inner.  k-major MM order (4 weight switches per
            # phase, banks finish staggered); last phase j-major with per-
            # bank copy+DMA so the tail is short.
            NPH = HB * DT
            for ph in range(NPH):
                h, d = divmod(ph, DT)
                d0 = d * 128
                last = ph == NPH - 1
                ot = op.tile([128, HW], dt_out, name=f"ot{ph}", tag="ot")
                pss = [
                    pp.tile([128, 512], mybir.dt.float32, tag="ps", name=f"ps_{h}_{d}_{j}")
                    for j in range(JH)
                ]
                # alternate output DMAs between the sync HWDGE queue and the
                # gpsimd SWDGE rings (POOL sequencer is otherwise idle) so
                # input and output streams don't serialize on one ring.
                # Scalar's sequencer is copy-only: a DMA issue between copies
                # delays the PSUM drain and back-pressures the PE.
                oeng = nc.gpsimd if ph % 2 == 0 else nc.sync
                if last and TAIL == "V":
                    # j0 drains as one 512 chunk on gpsimd; j1's two 256-col
                    # copies run CONCURRENTLY on ACT and DVE (PE is done, so
                    # a second PSUM reader can't throttle it), stores on
                    # sync + scalar in parallel
                    for k in range(KC):
                        nc.tensor.matmul(
                            pss[0][:],
                            m_sb[:, k, d0 : d0 + 128],
                            x_sb[(k, h)][:, 0:512],
                            start=(k == 0),
                            stop=(k == KC - 1),
                        )
                    nc.scalar.copy(ot[:, 0:512], pss[0][:])
                    nc.gpsimd.dma_start(
                        out=yt[d0 : d0 + 128, h * HW : h * HW + 512],
                        in_=ot[:, 0:512],
                    )
                    for k in range(KC):
                        nc.tensor.matmul(
                            pss[1][:],
                            m_sb[:, k, d0 : d0 + 128],
                            x_sb[(k, h)][:, 512:1024],
                            start=(k == 0),
                            stop=(k == KC - 1),
                        )
                    nc.vector.tensor_copy(out=ot[:, 768:1024], in_=pss[1][:, 256:512])
                    nc.scalar.copy(ot[:, 512:768], pss[1][:, 0:256])
                    nc.sync.dma_start(
                        out=yt[d0 : d0 + 128, h * HW + 512 : h * HW + 768],
                        in_=ot[:, 512:768],
                    )
                    nc.scalar.dma_start(
                        out=yt[d0 : d0 + 128, h * HW + 768 : h * HW + 1024],
                        in_=ot[:, 768:1024],
                    )
                elif last and TAIL == "S":
                    # split final phase: j0 drains as one 512-col chunk on
                    # gpsimd while j1 computes; j1 ends with two 256-col
                    # copies whose stores ride sync + scalar in parallel
                    for k in range(KC):
                        nc.tensor.matmul(
                            pss[0][:],
                            m_sb[:, k, d0 : d0 + 128],
                            x_sb[(k, h)][:, 0:512],
                            start=(k == 0),
                            stop=(k == KC - 1),
                        )
                    nc.scalar.copy(ot[:, 0:512], pss[0][:])
                    nc.gpsimd.dma_start(
                        out=yt[d0 : d0 + 128, h * HW : h * HW + 512],
                        in_=ot[:, 0:512],
                    )
                    for k in range(KC):
                        nc.tensor.matmul(
                            pss[1][:],
                            m_sb[:, k, d0 : d0 + 128],
                            x_sb[(k, h)][:, 512:1024],
                            start=(k == 0),
                            stop=(k == KC - 1),
                        )
                    nc.scalar.copy(ot[:, 512:768], pss[1][:, 0:256])
                    nc.sync.dma_start(
                        out=yt[d0 : d0 + 128, h * HW + 512 : h * HW + 768],
                        in_=ot[:, 512:768],
                    )
                    nc.scalar.copy(ot[:, 768:1024], pss[1][:, 256:512])
                    nc.scalar.dma_start(
                        out=yt[d0 : d0 + 128, h * HW + 768 : h * HW + 1024],
                        in_=ot[:, 768:1024],
                    )
                elif last:
                    # final phase: quarter-granularity copies, each quarter's
                    # store on a DIFFERENT engine's queue (sync x2, gpsimd,
                    # scalar-as-its-last-instruction) so the four issues
                    # don't serialize on one sequencer after the last MM
                    q_eng = [nc.sync, nc.gpsimd, nc.sync, None]
                    tail_dma = []
                    for j in range(JH):
                        for k in range(KC):
                            nc.tensor.matmul(
                                pss[j][:],
                                m_sb[:, k, d0 : d0 + 128],
                                x_sb[(k, h)][:, j * 512 : (j + 1) * 512],
                                start=(k == 0),
                                stop=(k == KC - 1),
                            )
                        for q in range(2):
                            c0 = j * 512 + q * 256
                            nc.scalar.copy(ot[:, c0 : c0 + 256], pss[j][:, q * 256 : (q + 1) * 256])
                            eng = q_eng[j * 2 + q]
                            args = dict(
                                out=yt[d0 : d0 + 128, h * HW + c0 : h * HW + c0 + 256],
                                in_=ot[:, c0 : c0 + 256],
                            )
                            if eng is None:
                                tail_dma.append(args)  # issue on ACT after all copies
                            else:
                                eng.dma_start(**args)
                    for args in tail_dma:
                        nc.scalar.dma_start(**args)
                else:
                    for k in range(KC):
                        for j in range(JH):
                            nc.tensor.matmul(
                                pss[j][:],
                                m_sb[:, k, d0 : d0 + 128],
                                x_sb[(k, h)][:, j * 512 : (j + 1) * 512],
                                start=(k == 0),
                                stop=(k == KC - 1),
                            )
                    for j in range(JH):
                        nc.scalar.copy(ot[:, j * 512 : (j + 1) * 512], pss[j][:])
                    oeng.dma_start(
                        out=yt[d0 : d0 + 128, h * HW : (h + 1) * HW], in_=ot[:]
                    )
    nc.compile()
    return nc


def _fold_m(W_v, s_p, W_p, beta_p, W_o):
    """Host-side constant folding of the tiny parameter tensors into M."""
    W_v = np.asarray(W_v, dtype=np.float64)
    s_p = np.asarray(s_p, dtype=np.float64)
    W_p = np.asarray(W_p, dtype=np.float64)
    beta_p = np.asarray(beta_p, dtype=np.float64)
    W_o = np.asarray(W_o, dtype=np.float64)
    phi = np.einsum("h,dhc,d->hc", s_p, W_p, beta_p)
    norma = np.linalg.norm(phi, axis=1)  # [h]
    M = np.einsum("dhc,h,hce->de", W_v, norma, W_o)  # [512, 512]
    return M.astype(np.float32)


_prog_cache = {}
_last_in_maps = None  # kept for test.py profiling reuse
_last_result = None


def _run(in_maps, token, **kwargs):
    if token not in _prog_cache:
        _prog_cache[token] = _build(token)
    return run_bass_kernel_spmd(_prog_cache[token], in_maps, list(range(N_CORES)), **kwargs)


def kernel(x, W_v, s_p, c_p, W_p, W_A, W_o, beta_p, beta_i_p, **_unused):
    global _last_in_maps, _last_result
    token = COMPUTE_DTYPE
    np_dt = _np_dtype(token)

    x = np.asarray(x, dtype=np.float32)
    M = _fold_m(W_v, s_p, W_p, beta_p, W_o)

    # fp16 path: scale M by an exact power of two so M entries and y values
    # sit in fp16 normal range; undo on the host after the run
    out_unscale = 1.0
    if token == "fp16":
        amax = float(np.abs(M).max())
        if amax > 0:
            e = int(np.floor(-np.log2(amax)))
            M = M * np.float32(2.0**e)
            out_unscale = 2.0**-e

    B, N, Dd = x.shape
    assert B * N == ROWS and Dd == D, (x.shape,)

    mmc = np.ascontiguousarray(M.reshape(KC, 128, D).transpose(1, 0, 2)).astype(np_dt)
    xf = x.reshape(ROWS, D)

    in_maps = []
    for c in range(N_CORES):
        sh = xf[c * RPC : (c + 1) * RPC]               # [4096, 512]
        xT = sh.T.astype(np_dt)                        # [512, 4096]
        # [KC, 128, HB, HW] -> [KC, HB, 128, HW], each quarter contiguous
        xs = np.ascontiguousarray(
            xT.reshape(KC, 128, HB, HW).transpose(0, 2, 1, 3)
        )
        in_maps.append({"xt": xs, "mm": mmc})

    _last_in_maps = in_maps
    res = _run(in_maps, token)
    _last_result = res
    out = np.empty((ROWS, D), dtype=np.float32)
    for c in range(N_CORES):
        yc = res.results[c]["yt"].astype(np.float32)
        if out_unscale != 1.0:
            yc *= np.float32(out_unscale)
        out[c * RPC : (c + 1) * RPC] = yc.T
    return out.reshape(B, N, D)


if __name__ == "__main__":
    # smoke test with random data
    rng = np.random.default_rng(0)
    x = rng.standard_normal((8, 4096, 512)).astype(np.float32)
    W_v = rng.standard_normal((512, 8, 64)).astype(np.float32) * 0.01
    s_p = np.ones((8,), np.float32)
    c_p = np.ones((8,), np.float32)
    W_p = rng.standard_normal((512, 8, 64)).astype(np.float32) * 0.01
    W_A = rng.standard_normal((256, 64)).astype(np.float32)
    W_o = rng.standard_normal((8, 64, 512)).astype(np.float32) * 0.01
    beta_p = rng.standard_normal((512,)).astype(np.float32) * 1e-5
    beta_i_p = rng.standard_normal((4096, 512)).astype(np.float32) * 1e-5
    out = kernel(x, W_v=W_v, s_p=s_p, c_p=c_p, W_p=W_p, W_A=W_A, W_o=W_o,
                 beta_p=beta_p, beta_i_p=beta_i_p)
    M = _fold_m(W_v, s_p, W_p, beta_p, W_o)
    exp = (x.reshape(-1, 512).astype(np.float64) @ M.astype(np.float64)).reshape(8, 4096, 512)
    err = np.abs(out - exp).max() / (np.abs(exp).max() + 1e-30)
    print("smoke rel err:", err)

